# revision 20
# baseline (speedup 1.0000x reference)
"""CapsuleLayer (dynamic routing) Trainium2 kernel, v3.

Math (see reference): u_hat[b,j,n,o] = sum_i x[b,n,i] W[j,n,i,o]; 3 routing
iterations of softmax-over-j (j=2 -> sigmoid of logit diff) + squash.

Design (n sharded over 8 cores, 90 chunks of 128 n per core):
  - s-sums via chunk-diagonal matmuls, split M=64 with alternating
    tile_position columns (0,0)/(0,64) so LDWEIGHTS of one half overlaps
    the other half's MATMUL. PSUM [128,128]; diagonal [16,32] blocks are
    summed as four 32-aligned [32,64] slabs -> [32,64] partial; the
    odd/even fold happens after the AllReduce (via DRAM reads) or on the
    host for the final sweep.
  - logit pass per 512-n tile: z via 2 matmuls (lhsT = I4 (x) vT bf16,
    rhs = W4 fp8-e3m4), xz on DVE (x8 fp8), d via oneD matmuls (M=64
    alternating), one batched Sigmoid (ACT) -> w [128n,(k,b)] bf16,
    y = w*x via one broadcast mul per tile (DVE, some tiles on GpSimd).
  - squash computed entirely on DVE (quake rsqrt + 2 Newton steps; v only
    feeds routing logits, the final v2 is squashed on the host), so the
    ACT sigmoid table never gets swapped out.
  - AllReduce via CC stream; its bounce DMAs ride the sync queue which
    carries only early XW loads. W4/x8 ride scalar/gpsimd queues.
"""
import sys

sys.path.insert(0, "/opt/trn_rl_repo")

import numpy as np
import ml_dtypes

BF16 = ml_dtypes.bfloat16
FP8 = ml_dtypes.float8_e3m4  # TRN float8e3: 4 mantissa bits, max ~15.5
N_CORES = 8
B = 16
NIN = 91392
DI = 8
DO = 16
NC_N = NIN // N_CORES  # 11424
CHUNKS = 90
NCP = CHUNKS * 128  # 11520
GS = 8  # XW group size (chunks)
EPS = 1e-7

# z tiles: 22 of 512 cols + 1 of 256
ZT = [(t * 512, 512) for t in range(22)] + [(22 * 512, 256)]

_CACHE = {}


def _patch_walrus_flags():
    """No-op: walrus' ldw-opt rejects bass-emitted InstLdweights."""


def host_prep(x, W, n_cores=N_CORES):
    n_per = x.shape[1] // n_cores  # 11424
    oneD = np.zeros((128, 16), dtype=BF16)
    for i in range(DI):
        for b in range(B):
            oneD[i * 16 + b, b] = 1.0
    in_maps = []
    for c in range(n_cores):
        xc = np.zeros((B, NCP, DI), dtype=np.float32)
        Wc = np.zeros((2, NCP, DI, DO), dtype=np.float32)
        xc[:, :n_per] = x[:, c * n_per : (c + 1) * n_per]
        Wc[:, :n_per] = W[:, c * n_per : (c + 1) * n_per]
        # xs[n128, c, (i,b)] ; ws[n128, c, (i,(j,o))]
        xs = (
            xc.reshape(B, CHUNKS, 128, DI).transpose(2, 1, 3, 0).reshape(128, CHUNKS, 128)
        ).astype(BF16)
        ws = (
            Wc.reshape(2, CHUNKS, 128, DI, DO)
            .transpose(2, 1, 3, 0, 4)
            .reshape(128, CHUNKS, 256)
        ).astype(BF16)
        cols = []
        for g0 in range(0, CHUNKS, GS):
            g1 = min(g0 + GS, CHUNKS)
            cols.append(xs[:, g0:g1].reshape(128, -1))
            cols.append(ws[:, g0:g1].reshape(128, -1))
        XW = np.ascontiguousarray(np.concatenate(cols, axis=1))
        # W4[(il,j,o), (H, n)] = W[j, n, H*4+il, o]
        W4 = np.ascontiguousarray(
            Wc.reshape(2, NCP, 2, 4, DO).transpose(3, 0, 4, 2, 1).reshape(128, 2 * NCP)
        ).astype(FP8)
        # x8[(i,b), n] = x[b,n,i]
        x8 = np.ascontiguousarray(xc.transpose(2, 0, 1).reshape(128, NCP)).astype(FP8)
        in_maps.append({"XW": XW, "W4": W4, "x8": x8, "oneD": oneD})
    return in_maps


def build_kernel(num_devices=N_CORES):
    from contextlib import ExitStack

    import concourse.bacc as bacc
    import concourse.tile as tile
    from concourse import mybir

    DT = mybir.dt.bfloat16
    F8 = mybir.dt.float8e3
    F32 = mybir.dt.float32
    U32 = mybir.dt.uint32
    AF = mybir.ActivationFunctionType
    OP = mybir.AluOpType

    ngrp = (CHUNKS + GS - 1) // GS  # 12 (last group has 2 chunks)
    xw_cols = CHUNKS * 384  # 34560

    nc = bacc.Bacc(
        "TRN2", target_bir_lowering=False, debug=False, num_devices=num_devices
    )
    xw_in = nc.declare_dram_parameter("XW", [128, xw_cols], DT, isOutput=False)
    w4_in = nc.declare_dram_parameter("W4", [128, 2 * NCP], F8, isOutput=False)
    x8_in = nc.declare_dram_parameter("x8", [128, NCP], F8, isOutput=False)
    oned_in = nc.declare_dram_parameter("oneD", [128, 16], DT, isOutput=False)
    t2_out = nc.declare_dram_parameter("t2", [32, 64], F32, isOutput=True)
    s0g_out = nc.declare_dram_parameter("s0g", [16, 32], F32, isOutput=True)

    ar_bufs = []
    for k in range(3):
        ar_bufs.append(
            (
                nc.dram_tensor(f"ar_in{k}", [32, 64], F32),
                nc.dram_tensor(f"ar_out{k}", [32, 64], F32, addr_space="Shared"),
            )
        )

    # W4/x8 park slices by z-tile groups (4,4,4,4,4,3 tiles)
    w4_slices = [(s * 2048, 2048) for s in range(5)] + [(10240, NCP - 10240)]

    with tile.TileContext(nc) as tc, ExitStack() as ctx:
        park = ctx.enter_context(tc.tile_pool(name="park", bufs=1))
        ps_acc = ctx.enter_context(tc.tile_pool(name="ps_acc", bufs=1, space="PSUM"))
        ps_z = ctx.enter_context(tc.tile_pool(name="ps_z", bufs=2, space="PSUM"))
        ps_d = ctx.enter_context(tc.tile_pool(name="ps_d", bufs=2, space="PSUM"))
        work = ctx.enter_context(tc.tile_pool(name="work", bufs=3))
        ypool = ctx.enter_context(tc.tile_pool(name="ypool", bufs=len(ZT)))
        small = ctx.enter_context(tc.tile_pool(name="small", bufs=8))

        # ---- bulk loads ----
        # sync carries only XW (drains early) so the AR bounce DMAs queued
        # on it later fire promptly; scalar/gpsimd carry the W4/x8 tail,
        # consumed slice-paced by pass 1 well after AR0.
        xw_engs = [nc.sync, nc.scalar, nc.gpsimd]
        xw_t = []
        off = 0
        for g in range(ngrp):
            gsz = min(GS, CHUNKS - g * GS)
            w = gsz * 384
            t = park.tile([128, w], DT, tag=f"xw{g}")
            xw_engs[g % 3].dma_start(t[:], xw_in[:, off : off + w])
            xw_t.append((t, gsz))
            off += w
        oneD = park.tile([128, 16], DT, tag="oneD")
        nc.sync.dma_start(oneD[:], oned_in[:])
        w4_view = w4_in[:].rearrange("p (h n) -> p h n", h=2)
        w4_t = []
        x8_t = []
        for k, (n0, ncols) in enumerate(w4_slices):
            eng = nc.scalar if k % 2 == 0 else nc.gpsimd
            t = park.tile([128, 2 * ncols], F8, tag=f"w4_{k}")
            eng.dma_start(
                t[:].rearrange("p (h n) -> p h n", h=2),
                w4_view[:, :, n0 : n0 + ncols],
            )
            w4_t.append(t)
            tx = park.tile([128, ncols], F8, tag=f"x8_{k}")
            eng.dma_start(tx[:], x8_in[:, n0 : n0 + ncols])
            x8_t.append(tx)

        # pre-zeroed skeletons for the per-pass stationaries
        vtp = park.tile([32, 32], F32, tag="vtp")
        nc.vector.memset(vtp[:], 0.0)
        s4sk = {}
        for it in (1, 2):
            s4t = park.tile([128, 64], DT, tag=f"s4_{it}", name=f"s4_{it}")
            nc.gpsimd.memset(s4t[:], 0.0)
            s4sk[it] = s4t
        # per-(b,j) sign/scale columns for baking the j=1 minus sign into v
        sgn1 = park.tile([16, 2], F32, tag="sgn1")  # [0.5, -0.5] (stage A)
        sgn2 = park.tile([16, 2], F32, tag="sgn2")  # [1, -1]
        nc.vector.memset(sgn1[:, 0:1], 0.5)
        nc.vector.memset(sgn1[:, 1:2], -0.5)
        nc.vector.memset(sgn2[:, 0:1], 1.0)
        nc.vector.memset(sgn2[:, 1:2], -1.0)

        def xs_cols(c, w):
            g, lc = c // GS, c % GS
            t, gsz = xw_t[g]
            return t[:, lc * 128 : lc * 128 + w]

        def ws_chunk(c):
            g, lc = c // GS, c % GS
            t, gsz = xw_t[g]
            off = gsz * 128 + lc * 256
            return t[:, off : off + 256]

        def w4_slice(H, n0, ncols):
            s = n0 // 2048
            loc = n0 - w4_slices[s][0]
            return w4_t[s][:].rearrange("p (h n) -> p h n", h=2)[
                :, H, loc : loc + ncols
            ]

        def x8_slice(n0, ncols):
            s = n0 // 2048
            loc = n0 - w4_slices[s][0]
            return x8_t[s][:, loc : loc + ncols]

        def sweep_mms(ps, c, lhs128, last):
            """Two M=64 chunk-diagonal matmuls with alternating columns."""
            for h in (0, 1):
                nc.tensor.matmul(
                    ps[h * 64 : h * 64 + 64, :],
                    lhs128[:, h * 64 : h * 64 + 64],
                    ws_chunk(c)[:, h * 128 : h * 128 + 128],
                    start=(c == 0),
                    stop=last,
                    tile_position=(0, h * 64),
                    skip_group_check=True,
                )

        def diag_extract(ps, tag):
            """[128,128] sweep PSUM -> [32,64] slab (true total needs the
            odd/even fold: slab[0:16,0:32] + slab[16:32,32:64])."""
            slabs = [
                ps[0:32, 0:64],
                ps[32:64, 64:128],
                ps[64:96, 0:64],
                ps[96:128, 64:128],
            ]
            a = small.tile([32, 64], F32, tag=f"dxa0_{tag}")
            nc.vector.tensor_copy(a[:], slabs[0])
            nxt = small.tile([32, 64], F32, tag=f"dxa1_{tag}")
            nc.vector.tensor_add(nxt[:], a[:], slabs[1])
            a2 = small.tile([32, 64], F32, tag=f"dxa2_{tag}")
            nc.vector.tensor_add(a2[:], nxt[:], slabs[2])
            out = small.tile([32, 64], F32, tag=f"dxo_{tag}")
            nc.vector.tensor_add(out[:], a2[:], slabs[3])
            return out

        def squash_pm(s_tile, sgn, tag):
            """vpm = squash(scale*s) with the j=1 sign flip baked in.

            sgn is a [16,2] per-(b,j) column of +/-scale. All on DVE: quake
            rsqrt + 2 Newton steps (v only feeds routing logits).
            """
            sq = small.tile([16, 32], F32, tag=f"sq_{tag}")
            nc.vector.tensor_mul(sq[:], s_tile[:], s_tile[:])
            sn = small.tile([16, 2], F32, tag=f"sn_{tag}")
            nc.vector.tensor_reduce(
                sn[:],
                sq[:].rearrange("p (j o) -> p j o", j=2),
                mybir.AxisListType.X,
                mybir.AluOpType.add,
            )
            # a = scale^2*sn + eps ; scale is baked via sgn (sgn^2 = scale^2)
            sc2 = small.tile([16, 2], F32, tag=f"sc2_{tag}")
            nc.vector.tensor_mul(sc2[:], sgn[:], sgn[:])
            sns = small.tile([16, 2], F32, tag=f"sns_{tag}")
            nc.vector.tensor_mul(sns[:], sn[:], sc2[:])
            # rs = 1/sqrt(sns+eps) via ACT Sqrt + DVE reciprocal (the
            # sqrt-table load hides in ACT idle time during the AllReduce)
            epst = small.tile([16, 1], F32, tag=f"epst_{tag}")
            nc.vector.memset(epst[:], EPS)
            sr = small.tile([16, 2], F32, tag=f"sr_{tag}")
            nc.scalar.activation(sr[:], sns[:], AF.Sqrt, bias=epst[:])
            rs = small.tile([16, 2], F32, tag=f"rs_{tag}")
            nc.vector.reciprocal(rs[:], sr[:])
            # f_pm = sgn * rsqrt * sns/(1+sns)
            den = small.tile([16, 2], F32, tag=f"den_{tag}")
            nc.vector.tensor_scalar_add(den[:], sns[:], 1.0)
            rec = small.tile([16, 2], F32, tag=f"rec_{tag}")
            nc.vector.reciprocal(rec[:], den[:])
            nr = small.tile([16, 2], F32, tag=f"nr_{tag}")
            nc.vector.tensor_mul(nr[:], sns[:], rec[:])
            f1 = small.tile([16, 2], F32, tag=f"f1_{tag}")
            nc.vector.tensor_mul(f1[:], rs[:], nr[:])
            fpm = small.tile([16, 2], F32, tag=f"fpm_{tag}")
            nc.vector.tensor_mul(fpm[:], f1[:], sgn[:])
            v = small.tile([16, 32], F32, tag=f"v_{tag}")
            nc.vector.tensor_mul(
                v[:].rearrange("p (j o) -> p j o", j=2),
                s_tile[:].rearrange("p (j o) -> p j o", j=2),
                fpm[:].unsqueeze(2).broadcast_to([16, 2, 16]),
            )
            return v

        def all_reduce(src_slab, idx):
            """SBUF [32,64] slab partial -> SBUF [16,32] folded global sum."""
            a_in, a_out = ar_bufs[idx]
            nc.sync.dma_start(a_in[:], src_slab[:])
            nc.gpsimd.collective_compute(
                "AllReduce",
                mybir.AluOpType.add,
                replica_groups=[list(range(num_devices))],
                ins=[a_in[:]],
                outs=[a_out[:]],
            )
            g1 = small.tile([16, 32], F32, tag=f"arg1_{idx}")
            g2 = small.tile([16, 32], F32, tag=f"arg2_{idx}")
            nc.sync.dma_start(g1[:], a_out[0:16, 0:32])
            nc.sync.dma_start(g2[:], a_out[16:32, 32:64])
            g = small.tile([16, 32], F32, tag=f"arg{idx}")
            nc.vector.tensor_add(g[:], g1[:], g2[:])
            return g

        # ---- stage A: t0[b,(j,o)] = sum_{n,i} x W (chunk-diagonal sweep) ----
        stA = ps_acc.tile([128, 128], F32, tag="stA")
        for c in range(CHUNKS):
            sweep_mms(stA, c, xs_cols(c, 128), last=(c == CHUNKS - 1))
        t0p = diag_extract(stA, "a")
        t0g = all_reduce(t0p, 0)
        vpm0 = squash_pm(t0g, sgn1, "v0")

        def routing_pass(vpm, it):
            # vT[(j,o), b] from the signed v via one copy + block transpose
            nc.vector.tensor_copy(vtp[0:16, :], vpm[:])
            vT = work.tile([32, 32], F32, tag="vT")
            nc.vector.transpose(vT[:], vtp[:])
            s4 = s4sk[it]
            for gg in range(4):
                nc.scalar.copy(
                    s4[gg * 32 : gg * 32 + 32, gg * 16 : gg * 16 + 16],
                    vT[0:32, 0:16],
                )

            ytiles = []
            for ti, (n0, nt) in enumerate(ZT):
                nk = nt // 128
                z_ps = ps_z.tile([128, 512], F32, tag="z")
                for H in (0, 1):
                    nc.tensor.matmul(
                        z_ps[H * 64 : H * 64 + 64, :nt],
                        s4[:, 0:64],
                        w4_slice(H, n0, nt),
                        start=True,
                        stop=True,
                        tile_position=(0, H * 64),
                        skip_group_check=True,
                    )
                xz = work.tile([128, 512], DT, tag="xz")
                nc.vector.tensor_mul(xz[:, :nt], z_ps[:, :nt], x8_slice(n0, nt))
                d_ps = ps_d.tile([128, 64], F32, tag="d")
                for k in range(nk):
                    for h in (0, 1):
                        nc.tensor.matmul(
                            d_ps[h * 64 : h * 64 + 64, k * 16 : k * 16 + 16],
                            xz[:, k * 128 + h * 64 : k * 128 + h * 64 + 64],
                            oneD[:],
                            start=True,
                            stop=True,
                            tile_position=(0, h * 64),
                            skip_group_check=True,
                        )
                w4b = work.tile([128, 64], DT, tag="w4b")
                nc.scalar.activation(
                    w4b[:, : nk * 16], d_ps[:, : nk * 16], AF.Sigmoid
                )
                y = ypool.tile([128, 512], DT, tag="y")
                yeng = nc.gpsimd if ti % 2 == 1 else nc.vector
                yeng.tensor_mul(
                    y[:, :nt].rearrange("p (k i b) -> p k i b", k=nk, i=8),
                    xs_cols(4 * ti, nt).rearrange("p (k i b) -> p k i b", k=nk, i=8),
                    w4b[:, : nk * 16]
                    .rearrange("p (k b) -> p k b", k=nk)
                    .unsqueeze(2)
                    .broadcast_to([128, nk, 8, 16]),
                )
                ytiles.append(y)

            stP = ps_acc.tile([128, 128], F32, tag=f"stP{it}")
            for c in range(CHUNKS):
                y = ytiles[c // 4]
                sweep_mms(
                    stP,
                    c,
                    y[:, (c % 4) * 128 : (c % 4) * 128 + 128],
                    last=(c == CHUNKS - 1),
                )
            return diag_extract(stP, f"i{it}")

        # ---- iteration 1 ----
        t1p = routing_pass(vpm0, 1)
        t1g = all_reduce(t1p, 1)
        s1 = small.tile([16, 32], F32, tag="s1")
        nc.vector.tensor_copy(s1[:, 0:16], t1g[:, 0:16])
        nc.vector.tensor_sub(s1[:, 16:32], t0g[:, 16:32], t1g[:, 16:32])
        vpm1 = squash_pm(s1, sgn2, "v1")
        vacc2 = small.tile([16, 32], F32, tag="vacc2")
        nc.vector.tensor_add(vacc2[:], vpm0[:], vpm1[:])

        # ---- iteration 2 (partials out; host combines) ----
        t2p = routing_pass(vacc2, 2)
        nc.sync.dma_start(t2_out[:], t2p[:])
        nc.sync.dma_start(s0g_out[:], t0g[:])

    nc.compile()
    return nc


def _squash_np(s):
    sn = np.sum(s * s, axis=-1, keepdims=True)
    return sn / (1.0 + sn) / np.sqrt(sn + EPS) * s


def finish_host(results):
    """Combine per-core (t2 slab, s0g) partials into v2 [16,2,16]."""
    t2s = sum(np.asarray(r["t2"], dtype=np.float64) for r in results)
    t2 = t2s[0:16, 0:32] + t2s[16:32, 32:64]
    s0g = np.asarray(results[0]["s0g"], dtype=np.float64)
    s2 = np.empty((16, 2, 16), dtype=np.float64)
    s2[:, 0, :] = t2[:, 0:16]
    s2[:, 1, :] = s0g[:, 16:32] - t2[:, 16:32]
    return _squash_np(s2).astype(np.float32)


def run(x, W, **spmd_kwargs):
    from concourse.bass_utils import run_bass_kernel_spmd

    _patch_walrus_flags()
    x = np.asarray(x, dtype=np.float32)
    W = np.asarray(W, dtype=np.float32)
    in_maps = host_prep(x, W)
    key = "nc_v3"
    if key not in _CACHE:
        _CACHE[key] = build_kernel()
    nc = _CACHE[key]
    res = run_bass_kernel_spmd(nc, in_maps, list(range(N_CORES)), **spmd_kwargs)
    return finish_host(res.results), res


def kernel(x, W):
    return run(x, W)[0]
